# revision 28
# baseline (speedup 1.0000x reference)
"""Trainium2 Bass kernel for MHA with RoPE (dense transformer block).

Problem shapes: h [1, 4096, 1024], 16 heads x 64 dim, full (non-causal)
softmax attention, post-softmax all-ones mask (identity), torch-Linear
projections with bias.

Sharding: head-parallel across 8 cores (2 heads/core). v2 design:

Prologue (projections, ~50us):
  - q/k/v projections from fp16 hT with weights stationary.
  - RoPE via PE rotation matmul: qs = R*(q+b) where R is the
    rotate-half permutation as an fp16 [128,128] matrix; then on DVE
    q' = (q+b) o cos + qs o sin (bias folded into ACT staging copy).
  - v transposed to key-major via fp16 PE transpose; bias bv is folded
    into bo on the host (softmax rows sum to 1 exactly).

Attention (PE/ACT/DVE balanced, ~185us):
  - per 512-query chunk x 32 key tiles: row-tiled QK pair (K=64 per
    head at tile_position (0,0)/(64,0), both heads concurrent, no
    zero-padding), one [128,1024] psum score tile per iteration.
  - exp alternates between ACT (table exp, scale=1/8) and DVE
    (Schraudolph: fp16 bits = int16(score*A + B), one tensor_scalar,
    bitcast free) halving the softmax wall time.
  - PV with a ones-row appended to v (denominator for free, M=65).
  - normalize with rec = exp(-ln(den) + ln 4096) on ACT (scaled to
    stay in fp16 normal range; host divides by 4096), broadcast via
    K=1 ones matmul, applied by DVE.
  - o_proj partials DMA'd straight from PSUM to DRAM fp32.

Host sums the 8 partial outputs, divides by 4096, adds bo + wo@bv.
"""

import math

import numpy as np

HIDDEN = 1024
HEADS = 16
HEAD_DIM = 64
SEQ = 4096
NCORES = 8
FPC = 128  # features per core = 2 heads x 64

# Schraudolph fp16-exp constants: fp16bits(e^(s/8)) ~= int16(s*EXP_A + EXP_B)
EXP_A = 1024.0 * 0.125 / math.log(2.0)
EXP_B = 15360.0 - 44.0

_NC_CACHE = {}


def _build_nc(S=SEQ):
    import concourse.bass as bass
    import concourse.tile as tile
    from concourse import mybir
    from contextlib import ExitStack

    f32 = mybir.dt.float32
    f16 = mybir.dt.float16
    i16 = mybir.dt.int16
    Exp = mybir.ActivationFunctionType.Exp
    Ln = mybir.ActivationFunctionType.Ln
    Ident = mybir.ActivationFunctionType.Identity
    Mult = mybir.AluOpType.mult
    Add = mybir.AluOpType.add

    D = HEAD_DIM
    HID = HIDDEN
    KT = HID // 128  # hidden contraction tiles
    PC = 512         # projection seq chunk
    NPC = S // PC
    CH = 512         # attention query chunk
    NCH = S // CH
    SK = S // 128    # key tiles
    PVLAG = 4        # software-pipeline lag of PV behind QK/exp

    nc = bass.Bass(trn_type="TRN2")

    hT = nc.dram_tensor("hT", [HID, S], f16, kind="ExternalInput")
    wqT = nc.dram_tensor("wqT", [HID, FPC], f16, kind="ExternalInput")
    wkT = nc.dram_tensor("wkT", [HID, FPC], f16, kind="ExternalInput")
    wvT = nc.dram_tensor("wvT", [HID, FPC], f16, kind="ExternalInput")
    bqk = nc.dram_tensor("bqk", [FPC, 2], f32, kind="ExternalInput")
    woT = nc.dram_tensor("woT", [FPC, HID], f16, kind="ExternalInput")
    cosT = nc.dram_tensor("cosT", [D, S], f16, kind="ExternalInput")
    sinT = nc.dram_tensor("sinT", [D, S], f16, kind="ExternalInput")
    out = nc.dram_tensor("out", [S, HID], f16, kind="ExternalOutput")

    hT3 = hT[:, :].rearrange("(ko p) s -> p ko s", p=128)

    # rotate-half as a matmul: qs = R @ q with R = blockdiag([[0,-I],[I,0]]).
    # matmul computes lhsT.T @ rhs so we feed R^T = blockdiag([[0,I],[-I,0]]).
    rotT_np = np.zeros((FPC, FPC), dtype=np.float16)
    for hh in range(2):
        o = hh * 64
        for j in range(32):
            rotT_np[o + 32 + j, o + j] = -1.0
            rotT_np[o + j, o + 32 + j] = 1.0
    rotT_dram = nc.inline_tensor(rotT_np, name="rotT")
    ident_np = np.eye(128, dtype=np.float16)
    ident_dram = nc.inline_tensor(ident_np, name="ident16")

    with tile.TileContext(nc) as tc, ExitStack() as top:
        sing = top.enter_context(tc.tile_pool(name="sing", bufs=1))

        # input DMAs are spread across the engines' DGE queues so the first
        # h chunk isn't serialized behind weights/tables on one queue: h
        # chunks own the sync queue, weights go via scalar, tables via
        # vector/gpsimd.
        wq_sb = sing.tile([128, KT, FPC], f16)
        wk_sb = sing.tile([128, KT, FPC], f16)
        wv_sb = sing.tile([128, KT, FPC], f16)
        nc.scalar.dma_start(wq_sb, wqT[:, :].rearrange("(ko p) f -> p ko f", p=128))
        nc.scalar.dma_start(wk_sb, wkT[:, :].rearrange("(ko p) f -> p ko f", p=128))
        nc.scalar.dma_start(wv_sb, wvT[:, :].rearrange("(ko p) f -> p ko f", p=128))
        wo_sb = sing.tile([FPC, HID], f16)
        nc.scalar.dma_start(wo_sb, woT[:, :])
        b_sb = sing.tile([FPC, 2], f32)
        nc.gpsimd.dma_start(b_sb, bqk[:, :])
        cos_sb = sing.tile([128, S], f16)
        sin_sb = sing.tile([128, S], f16)
        nc.gpsimd.dma_start(cos_sb[0:64, :], cosT[:, :])
        nc.gpsimd.dma_start(cos_sb[64:128, :], cosT[:, :])
        nc.gpsimd.dma_start(sin_sb[0:64, :], sinT[:, :])
        nc.gpsimd.dma_start(sin_sb[64:128, :], sinT[:, :])
        rot_sb = sing.tile([FPC, FPC], f16)
        nc.gpsimd.dma_start(rot_sb, rotT_dram[:, :])
        ident_sb = sing.tile([128, 128], f16)
        nc.gpsimd.dma_start(ident_sb, ident_dram[:, :])
        ones_sb = sing.tile([1, 64], f16)
        nc.vector.memset(ones_sb, 1.0)
        ln4096_sb = sing.tile([1, 1], f32)
        nc.vector.memset(ln4096_sb, math.log(4096.0))

        # warm the natural_log_exp table set during the prologue so the
        # first attention exp doesn't eat the ~2.7us ACT_TABLE_LOAD
        warm_sb = sing.tile([1, 1], f32)
        nc.scalar.activation(warm_sb, ln4096_sb, Ln)
        nc.scalar.activation(warm_sb, warm_sb, Exp)

        qT_sb = sing.tile([128, S], f16)
        kp_sb = sing.tile([128, S], f16)
        # v1[:, hh, i, 0:64] = v tile (keys-major); [.., 64] = ones row so the
        # PV matmul also produces the softmax denominator.
        v1_sb = sing.tile([128, 2, SK, 65], f16)
        nc.vector.memset(v1_sb, 1.0)
        ctx_sb = sing.tile([128, S], f16)

        # pt tiles span both phases: chunk 0's probabilities are computed
        # during the projection prologue (exp-ahead) and consumed by PV in
        # the attention phase; the ring is deep enough to hold one full
        # chunk (32) plus the next chunk's in-flight tiles.
        ptp = top.enter_context(tc.tile_pool(name="ptp", bufs=40))

        def emit_qk_exp(pool, pts_list, c, i):
            cs0 = c * CH
            ksl = slice(i * 128, (i + 1) * 128)
            csl = slice(cs0, cs0 + CH)
            ss = pool.tile([128, 2 * CH], f32, tag="ss", name=f"ss_{c}_{i}")
            for hh in range(2):
                psl = slice(hh * 64, hh * 64 + 64)
                nc.tensor.matmul(
                    ss[:, hh * CH : (hh + 1) * CH],
                    kp_sb[psl, ksl],
                    qT_sb[psl, csl],
                    start=True,
                    stop=True,
                )
            pt = ptp.tile([128, 2 * CH], f16, tag="pt", name=f"pt_{c}_{i}")
            # strict alternation: back-to-back same-engine exps serialize
            # and stall the ss ring
            if i % 2 == 0:
                nc.scalar.activation(pt, ss, Exp, scale=0.125)
            else:
                nc.vector.tensor_scalar(
                    pt[:, :].bitcast(i16), ss, EXP_A, EXP_B, Mult, Add
                )
            pts_list[i] = pt

        pts0 = [None] * SK

        # ---- projections + RoPE + v transpose (+ chunk-0 QK/exp) ----
        with ExitStack() as ph1:
            hp = ph1.enter_context(tc.tile_pool(name="hp", bufs=2))
            rp = ph1.enter_context(tc.tile_pool(name="rope", bufs=8))
            pps = ph1.enter_context(tc.tile_pool(name="pps", bufs=2, space="PSUM"))
            qsp = ph1.enter_context(tc.tile_pool(name="qsp", bufs=1, space="PSUM"))
            tps = ph1.enter_context(tc.tile_pool(name="tps", bufs=1, space="PSUM"))
            ssP = ph1.enter_context(tc.tile_pool(name="ssP", bufs=2, space="PSUM"))
            # Software-pipelined: each projection group's PE epilogue (the
            # rotation matmul / v transposes, which wait on an ACT staging
            # copy) is emitted under the NEXT group's matmul stream so the
            # PE never stalls on ACT latency.
            pend = []

            def rope_tail(ch, wi, stg, dst):
                ssl = slice(ch * PC, (ch + 1) * PC)
                qs = qsp.tile([128, PC], f32, tag="qs", name=f"qs_{ch}_{wi}")
                nc.tensor.matmul(qs, rot_sb, stg, start=True, stop=True)
                t1 = rp.tile([128, PC], f16, tag="t1", name=f"t1_{ch}_{wi}")
                nc.vector.tensor_mul(t1, stg, cos_sb[:, ssl])
                t2 = rp.tile([128, PC], f16, tag="t2", name=f"t2_{ch}_{wi}")
                nc.vector.tensor_mul(t2, qs, sin_sb[:, ssl])
                nc.vector.tensor_add(dst[:, ssl], t1, t2)

            def v_tail(ch, stgv):
                for st in range(PC // 128):
                    kti = ch * (PC // 128) + st
                    tp = tps.tile([128, 128], f16, tag="tp", name=f"tp_{ch}_{st}")
                    nc.tensor.transpose(
                        tp, stgv[:, st * 128 : (st + 1) * 128], ident_sb
                    )
                    nc.vector.tensor_copy(v1_sb[:, :, kti, 0:64], tp)

            for ch in range(NPC):
                ssl = slice(ch * PC, (ch + 1) * PC)
                h_sb = hp.tile([128, KT, PC], f16)
                nc.sync.dma_start(h_sb, hT3[:, :, ssl])
                for wi, (w_sb, dst) in enumerate(
                    [(wq_sb, qT_sb), (wk_sb, kp_sb), (wv_sb, None)]
                ):
                    ps = pps.tile([128, PC], f32, tag="ps", name=f"ps_{ch}_{wi}")
                    for k in range(KT):
                        nc.tensor.matmul(
                            ps,
                            w_sb[:, k, :],
                            h_sb[:, k, :],
                            start=(k == 0),
                            stop=(k == KT - 1),
                        )
                    if pend:
                        pend.pop(0)()
                    if dst is not None:
                        # stage with bias on ACT (psum -> fp16 sbuf)
                        stg = rp.tile([128, PC], f16, tag="stg", name=f"stg_{ch}_{wi}")
                        nc.scalar.activation(
                            stg, ps, Ident, bias=b_sb[:, wi : wi + 1]
                        )
                        pend.append(
                            lambda ch=ch, wi=wi, stg=stg, dst=dst: rope_tail(
                                ch, wi, stg, dst
                            )
                        )
                    else:
                        stgv = rp.tile([128, PC], f16, tag="stgv", name=f"stgv_{ch}")
                        nc.scalar.activation(stgv, ps, Ident)
                        pend.append(lambda ch=ch, stgv=stgv: v_tail(ch, stgv))
                # chunk-0 attention scores for the key tiles this proj chunk
                # just produced: the exp work rides the otherwise idle
                # ACT/DVE capacity of the prologue
                for i in range(4 * ch, 4 * ch + 4):
                    emit_qk_exp(ssP, pts0, 0, i)
            for w in pend:
                w()

        # ---- attention + o_proj ----
        # PV runs one full chunk behind QK/exp: chunk c's loop emits QK/exp
        # for chunk c and PV for chunk c-1 from the persisted pt ring, so the
        # epilogue (den -> rec -> normalize) of each chunk has a whole chunk
        # of slack before its cx slot is needed again, and chunk 0's QK/exp
        # were already emitted in the prologue.
        with ExitStack() as ph2:
            ssp = ph2.enter_context(tc.tile_pool(name="ssp", bufs=3, space="PSUM"))
            cxp = ph2.enter_context(tc.tile_pool(name="cxp", bufs=1, space="PSUM"))
            obp = ph2.enter_context(tc.tile_pool(name="obp", bufs=4))
            epp = ph2.enter_context(tc.tile_pool(name="epp", bufs=2))

            def emit_oproj_pair(c, sq):
                # one [128 q, 1024 hid] output row-block of chunk c's o_proj:
                # two matmuls into the two banks of one ss slot, one staging
                # copy, one DMA
                r0 = c * CH + sq * 128
                ops = ssp.tile([128, 2 * CH], f32, tag="ss", name=f"op_{c}_{sq}")
                for nz in range(2):
                    nc.tensor.matmul(
                        ops[:, nz * 512 : (nz + 1) * 512],
                        ctx_sb[:, r0 : r0 + 128],
                        wo_sb[:, nz * 512 : (nz + 1) * 512],
                        start=True,
                        stop=True,
                    )
                ob = obp.tile([128, 1024], f16, tag="ob", name=f"ob_{c}_{sq}")
                # psum->sbuf fp16 staging; ACT takes most (DVE exp is pricier)
                if sq % 4 == 1:
                    nc.vector.tensor_copy(ob, ops)
                else:
                    nc.scalar.activation(ob, ops, Ident)
                nc.sync.dma_start(out[r0 : r0 + 128, :], ob)

            def emit_warmer(tag, n=10):
                # dense burst of throwaway M=1 matmuls: HAM un-throttles only
                # after a sustained-busy window. Alternate between the two
                # banks of one ss slot so same-bank WAW doesn't serialize.
                wps = ssp.tile([128, 2 * CH], f32, tag="ss", name=f"warm_{tag}")
                for j in range(n):
                    nc.tensor.matmul(
                        wps[0:1, (j % 2) * CH : (j % 2) * CH + CH],
                        kp_sb[:, 0:1],
                        qT_sb[:, 0:CH],
                        start=True,
                        stop=True,
                        skip_group_check=True,
                    )

            def emit_pv(cx, pts_list, i):
                for hh in range(2):
                    nc.tensor.matmul(
                        cx[:, hh, :],
                        v1_sb[:, hh, i, :],
                        pts_list[i][:, hh * CH : (hh + 1) * CH],
                        start=(i == 0),
                        stop=(i == SK - 1),
                    )

            def emit_epilogue(c, cx):
                cs0 = c * CH
                # den -> Ln straight from psum while DVE stages the context;
                # both heads in single wide instructions
                lnb = epp.tile([1, 2 * CH], f32, tag="lnb", name=f"lnb_{c}")
                nc.scalar.activation(lnb, cx[64:65, :, :], Ln)
                stage = epp.tile([64, 2, CH], f16, tag="stage", name=f"stage_{c}")
                nc.vector.tensor_copy(stage, cx[0:64, :, :])
                rec = epp.tile([1, 2 * CH], f16, tag="rec", name=f"rec_{c}")
                # rec = 4096/den keeps fp16 in normal range; host undoes it
                nc.scalar.activation(rec, lnb, Exp, scale=-1.0, bias=ln4096_sb[:, :])
                rb = ssp.tile([128, 2 * CH], f32, tag="ss", name=f"rb_{c}")
                for hh in range(2):
                    nc.tensor.matmul(
                        rb[0:64, hh * CH : (hh + 1) * CH],
                        ones_sb,
                        rec[:, hh * CH : (hh + 1) * CH],
                        start=True,
                        stop=True,
                    )
                for hh in range(2):
                    hsl = slice(hh * 64, hh * 64 + 64)
                    nc.vector.tensor_mul(
                        ctx_sb[hsl, cs0 : cs0 + CH],
                        stage[:, hh, :],
                        rb[0:64, hh * CH : (hh + 1) * CH],
                    )

            pts_prev = pts0
            for c in range(1, NCH):
                cx = cxp.tile([65, 2, CH], f32, tag="cx", name=f"cx_{c - 1}")
                pts_cur = [None] * SK
                for i in range(SK):
                    emit_qk_exp(ssp, pts_cur, c, i)
                    emit_pv(cx, pts_prev, i)
                    if i == 10 or i == 20:
                        if c >= 2:
                            for sq in range(2 * (i == 20), 2 * (i == 20) + 2):
                                emit_oproj_pair(c - 2, sq)
                        else:
                            emit_warmer(f"w{c}_{i}")
                emit_epilogue(c - 1, cx)
                pts_prev = pts_cur
            # drain: PV + epilogue + o_proj of the last chunk
            cx = cxp.tile([65, 2, CH], f32, tag="cx", name=f"cx_{NCH - 1}")
            for i in range(SK):
                emit_pv(cx, pts_prev, i)
                if i == 10 or i == 20:
                    for sq in range(2 * (i == 20), 2 * (i == 20) + 2):
                        emit_oproj_pair(NCH - 2, sq)
            emit_epilogue(NCH - 1, cx)
            for sq in range(4):
                emit_oproj_pair(NCH - 1, sq)
    return nc


def _legalize_sync_waits(nc, max_waits=1):
    """Cap sync waits per instruction for this container's walrus build.

    The bundled walrus encodes a limited number of sync-wait commands per
    instruction ("Too many sync wait commands" codegen error), while Tile
    attaches one wait per logical processor where needed. An attached wait
    is equivalent to a standalone preceding wait on the same engine (that
    is exactly what raw-bass `wait_ge` emits: a pure-wait
    InstEventSemaphore), so hoist the excess waits onto EventSemaphore
    instructions inserted right before the offender.
    """
    from concourse import mybir

    n_fixed = 0
    for fn in nc.m.functions:
        for b in fn.blocks:
            insts = b.instructions
            idx = 0
            while idx < len(insts):
                inst = insts[idx]
                si = inst.sync_info
                waits = list(si.on_wait) if si and si.on_wait else []
                if len(waits) > max_waits:
                    updates = list(si.on_update) if si and si.on_update else []
                    pre, keep = waits[: -max_waits], waits[-max_waits:]
                    clones = []
                    for j, w in enumerate(pre):
                        clones.append(
                            mybir.InstEventSemaphore(
                                name=f"{inst.name}_sw{j}",
                                engine=inst.engine,
                                ins=[],
                                outs=[],
                                sync_info=mybir.SyncInfo(on_wait=[w], on_update=[]),
                            )
                        )
                    inst.sync_info = mybir.SyncInfo(on_wait=keep, on_update=updates)
                    for j, clone in enumerate(clones):
                        insts.insert(idx + j, clone)
                        try:
                            nc.inst_map[clone.name] = clone
                        except Exception:
                            pass
                    idx += len(clones)
                    n_fixed += 1
                idx += 1
    return n_fixed


MM_DT = "float16"


def get_nc(S=SEQ, mm_dt=MM_DT):
    key = S
    if key not in _NC_CACHE:
        nc = _build_nc(S)
        _legalize_sync_waits(nc)
        _NC_CACHE[key] = nc
    return _NC_CACHE[key]


def make_in_maps(h, cos, sin, wq, bq, wk, bk, wv, bv, wo):
    """Host-side shard prep. h [B,S,HID] -> per-core input dict."""
    f16 = np.float16
    h = np.asarray(h, dtype=np.float32)
    S = h.shape[1]
    hT = np.ascontiguousarray(h[0].T).astype(f16)  # [HID, S]
    cosT = np.ascontiguousarray(np.asarray(cos, np.float32).T).astype(f16)
    sinT = np.ascontiguousarray(np.asarray(sin, np.float32).T).astype(f16)
    wq = np.asarray(wq, dtype=np.float32)
    wk = np.asarray(wk, dtype=np.float32)
    wv = np.asarray(wv, dtype=np.float32)
    wo = np.asarray(wo, dtype=np.float32)
    bq = np.asarray(bq, dtype=np.float32)
    bk = np.asarray(bk, dtype=np.float32)
    in_maps = []
    for c in range(NCORES):
        fs = slice(c * FPC, (c + 1) * FPC)
        in_maps.append(
            {
                "hT": hT,
                "wqT": np.ascontiguousarray(wq[fs, :].T).astype(f16),
                "wkT": np.ascontiguousarray(wk[fs, :].T).astype(f16),
                "wvT": np.ascontiguousarray(wv[fs, :].T).astype(f16),
                "bqk": np.ascontiguousarray(
                    np.stack([bq[fs], bk[fs]], axis=1).astype(np.float32)
                ),
                "woT": np.ascontiguousarray(wo[:, fs].T).astype(f16),
                "cosT": cosT,
                "sinT": sinT,
            }
        )
    return in_maps


def kernel(h, mask, cos, sin, wq, bq, wk, bk, wv, bv, wo, bo, **_unused):
    # mask is all-ones per the problem spec; post-softmax where(mask==0) is a no-op.
    from concourse.bass_utils import run_bass_kernel_spmd

    h = np.asarray(h, dtype=np.float32)
    S = h.shape[1]
    nc = get_nc(S)
    in_maps = make_in_maps(h, cos, sin, wq, bq, wk, bk, wv, bv, wo)
    res = run_bass_kernel_spmd(nc, in_maps, core_ids=list(range(NCORES)))
    acc = np.zeros((S, HIDDEN), dtype=np.float64)
    for r in res.results:
        acc += r["out"].astype(np.float64)
    acc /= 4096.0
    bo_eff = np.asarray(bo, np.float64) + np.asarray(wo, np.float64) @ np.asarray(
        bv, np.float64
    )
    acc += bo_eff[None, :]
    return acc[None].astype(np.float32)


# revision 33
# speedup vs baseline: 1.0259x; 1.0259x over previous
"""Trainium2 Bass kernel for MHA with RoPE (dense transformer block).

Problem shapes: h [1, 4096, 1024], 16 heads x 64 dim, full (non-causal)
softmax attention, post-softmax all-ones mask (identity), torch-Linear
projections with bias.

Sharding: head-parallel across 8 cores (2 heads/core). v2 design:

Prologue (projections, ~50us):
  - q/k/v projections from fp16 hT with weights stationary.
  - RoPE via PE rotation matmul: qs = R*(q+b) where R is the
    rotate-half permutation as an fp16 [128,128] matrix; then on DVE
    q' = (q+b) o cos + qs o sin (bias folded into ACT staging copy).
  - v transposed to key-major via fp16 PE transpose; bias bv is folded
    into bo on the host (softmax rows sum to 1 exactly).

Attention (PE/ACT/DVE balanced, ~185us):
  - per 512-query chunk x 32 key tiles: row-tiled QK pair (K=64 per
    head at tile_position (0,0)/(64,0), both heads concurrent, no
    zero-padding), one [128,1024] psum score tile per iteration.
  - exp alternates between ACT (table exp, scale=1/8) and DVE
    (Schraudolph: fp16 bits = int16(score*A + B), one tensor_scalar,
    bitcast free) halving the softmax wall time.
  - PV with a ones-row appended to v (denominator for free, M=65).
  - normalize with rec = exp(-ln(den) + ln 4096) on ACT (scaled to
    stay in fp16 normal range; host divides by 4096), broadcast via
    K=1 ones matmul, applied by DVE.
  - o_proj partials DMA'd straight from PSUM to DRAM fp32.

Host sums the 8 partial outputs, divides by 4096, adds bo + wo@bv.
"""

import math

import numpy as np

HIDDEN = 1024
HEADS = 16
HEAD_DIM = 64
SEQ = 4096
NCORES = 8
FPC = 128  # features per core = 2 heads x 64

# Schraudolph fp16-exp constants: fp16bits(e^(s/8)) ~= int16(s*EXP_A + EXP_B)
EXP_A = 1024.0 * 0.125 / math.log(2.0)
EXP_B = 15360.0 - 44.0

_NC_CACHE = {}


def _build_nc(S=SEQ):
    import concourse.bass as bass
    import concourse.tile as tile
    from concourse import mybir
    from contextlib import ExitStack

    f32 = mybir.dt.float32
    f16 = mybir.dt.float16
    i16 = mybir.dt.int16
    Exp = mybir.ActivationFunctionType.Exp
    Ln = mybir.ActivationFunctionType.Ln
    Ident = mybir.ActivationFunctionType.Identity
    Mult = mybir.AluOpType.mult
    Add = mybir.AluOpType.add

    D = HEAD_DIM
    HID = HIDDEN
    KT = HID // 128  # hidden contraction tiles
    PC = 512         # projection seq chunk
    NPC = S // PC
    CH = 512         # attention query chunk
    NCH = S // CH
    SK = S // 128    # key tiles
    PVLAG = 4        # software-pipeline lag of PV behind QK/exp

    nc = bass.Bass(trn_type="TRN2")

    # host pre-arranges h and the q/k/v weights partition-major so every
    # input DMA is one contiguous descriptor per partition (the naive
    # "(ko p) s" rearrange yields 1024 scattered 256B descriptors and a
    # ~20us transfer)
    hR = nc.dram_tensor("hR", [128, NPC, KT, PC], f16, kind="ExternalInput")
    wqR = nc.dram_tensor("wqR", [128, KT, FPC], f16, kind="ExternalInput")
    wkR = nc.dram_tensor("wkR", [128, KT, FPC], f16, kind="ExternalInput")
    wvR = nc.dram_tensor("wvR", [128, KT, FPC], f16, kind="ExternalInput")
    bqk = nc.dram_tensor("bqk", [FPC, 2], f32, kind="ExternalInput")
    woT = nc.dram_tensor("woT", [FPC, HID], f16, kind="ExternalInput")
    cosT = nc.dram_tensor("cosT", [D, S], f16, kind="ExternalInput")
    sinT = nc.dram_tensor("sinT", [D, S], f16, kind="ExternalInput")
    out = nc.dram_tensor("out", [S, HID], f16, kind="ExternalOutput")

    # rotate-half as a matmul: qs = R @ q with R = blockdiag([[0,-I],[I,0]]).
    # matmul computes lhsT.T @ rhs so we feed R^T = blockdiag([[0,I],[-I,0]]).
    rotT_np = np.zeros((FPC, FPC), dtype=np.float16)
    for hh in range(2):
        o = hh * 64
        for j in range(32):
            rotT_np[o + 32 + j, o + j] = -1.0
            rotT_np[o + j, o + 32 + j] = 1.0
    rotT_dram = nc.inline_tensor(rotT_np, name="rotT")
    ident_np = np.eye(128, dtype=np.float16)
    ident_dram = nc.inline_tensor(ident_np, name="ident16")

    with tile.TileContext(nc) as tc, ExitStack() as top:
        sing = top.enter_context(tc.tile_pool(name="sing", bufs=1))

        # input DMAs are spread across the engines' DGE queues so the first
        # h chunk isn't serialized behind weights/tables on one queue: h
        # chunks own the sync queue, weights go via scalar, tables via
        # vector/gpsimd.
        wq_sb = sing.tile([128, KT, FPC], f16)
        wk_sb = sing.tile([128, KT, FPC], f16)
        wv_sb = sing.tile([128, KT, FPC], f16)
        nc.scalar.dma_start(wq_sb, wqR[:, :, :])
        nc.scalar.dma_start(wk_sb, wkR[:, :, :])
        nc.scalar.dma_start(wv_sb, wvR[:, :, :])
        cos_sb = sing.tile([128, S], f16)
        sin_sb = sing.tile([128, S], f16)
        nc.scalar.dma_start(cos_sb[0:64, :], cosT[:, :])
        nc.scalar.dma_start(sin_sb[0:64, :], sinT[:, :])
        nc.scalar.dma_start(cos_sb[64:128, :], cosT[:, :])
        nc.scalar.dma_start(sin_sb[64:128, :], sinT[:, :])
        wo_sb = sing.tile([FPC, HID], f16)
        nc.scalar.dma_start(wo_sb, woT[:, :])
        b_sb = sing.tile([FPC, 2], f32)
        nc.gpsimd.dma_start(b_sb, bqk[:, :])
        rot_sb = sing.tile([FPC, FPC], f16)
        nc.gpsimd.dma_start(rot_sb, rotT_dram[:, :])
        ident_sb = sing.tile([128, 128], f16)
        nc.gpsimd.dma_start(ident_sb, ident_dram[:, :])
        ones_sb = sing.tile([1, 64], f16)
        nc.vector.memset(ones_sb, 1.0)
        ln4096_sb = sing.tile([1, 1], f32)
        nc.vector.memset(ln4096_sb, math.log(4096.0))

        # warm the natural_log_exp table set during the prologue so the
        # first attention exp doesn't eat the ~2.7us ACT_TABLE_LOAD
        warm_sb = sing.tile([1, 1], f32)
        nc.scalar.activation(warm_sb, ln4096_sb, Ln)
        nc.scalar.activation(warm_sb, warm_sb, Exp)

        qT_sb = sing.tile([128, S], f16)
        kp_sb = sing.tile([128, S], f16)
        # v1[:, hh, i, 0:64] = v tile (keys-major); [.., 64] = ones row so the
        # PV matmul also produces the softmax denominator.
        v1_sb = sing.tile([128, 2, SK, 65], f16)
        nc.vector.memset(v1_sb, 1.0)
        ctx_sb = sing.tile([128, S], f16)

        # pt tiles span both phases: chunk 0's probabilities are computed
        # during the projection prologue (exp-ahead) and consumed by PV in
        # the attention phase; the ring is deep enough to hold one full
        # chunk (32) plus the next chunk's in-flight tiles.
        ptp = top.enter_context(tc.tile_pool(name="ptp", bufs=40))

        def emit_qk_exp(pool, pts_list, c, i):
            cs0 = c * CH
            ksl = slice(i * 128, (i + 1) * 128)
            csl = slice(cs0, cs0 + CH)
            ss = pool.tile([128, 2 * CH], f32, tag="ss", name=f"ss_{c}_{i}")
            for hh in range(2):
                psl = slice(hh * 64, hh * 64 + 64)
                nc.tensor.matmul(
                    ss[:, hh * CH : (hh + 1) * CH],
                    kp_sb[psl, ksl],
                    qT_sb[psl, csl],
                    start=True,
                    stop=True,
                )
            pt = ptp.tile([128, 2 * CH], f16, tag="pt", name=f"pt_{c}_{i}")
            # strict alternation: back-to-back same-engine exps serialize
            # and stall the ss ring
            if i % 2 == 0:
                nc.scalar.activation(pt, ss, Exp, scale=0.125)
            else:
                nc.vector.tensor_scalar(
                    pt[:, :].bitcast(i16), ss, EXP_A, EXP_B, Mult, Add
                )
            pts_list[i] = pt

        pts0 = [None] * SK

        # ---- projections + RoPE + v transpose (+ chunk-0 QK/exp) ----
        with ExitStack() as ph1:
            hp = ph1.enter_context(tc.tile_pool(name="hp", bufs=2))
            rp = ph1.enter_context(tc.tile_pool(name="rope", bufs=8))
            pps = ph1.enter_context(tc.tile_pool(name="pps", bufs=2, space="PSUM"))
            qsp = ph1.enter_context(tc.tile_pool(name="qsp", bufs=1, space="PSUM"))
            tps = ph1.enter_context(tc.tile_pool(name="tps", bufs=1, space="PSUM"))
            ssP = ph1.enter_context(tc.tile_pool(name="ssP", bufs=2, space="PSUM"))
            # Software-pipelined: each projection group's PE epilogue (the
            # rotation matmul / v transposes, which wait on an ACT staging
            # copy) is emitted under the NEXT group's matmul stream so the
            # PE never stalls on ACT latency.
            pend = []

            def rope_tail(ch, wi, stg, dst):
                ssl = slice(ch * PC, (ch + 1) * PC)
                qs = qsp.tile([128, PC], f32, tag="qs", name=f"qs_{ch}_{wi}")
                nc.tensor.matmul(qs, rot_sb, stg, start=True, stop=True)
                t1 = rp.tile([128, PC], f16, tag="t1", name=f"t1_{ch}_{wi}")
                nc.vector.tensor_mul(t1, stg, cos_sb[:, ssl])
                t2 = rp.tile([128, PC], f16, tag="t2", name=f"t2_{ch}_{wi}")
                nc.vector.tensor_mul(t2, qs, sin_sb[:, ssl])
                nc.vector.tensor_add(dst[:, ssl], t1, t2)

            def v_tail(ch, stgv):
                for st in range(PC // 128):
                    kti = ch * (PC // 128) + st
                    tp = tps.tile([128, 128], f16, tag="tp", name=f"tp_{ch}_{st}")
                    nc.tensor.transpose(
                        tp, stgv[:, st * 128 : (st + 1) * 128], ident_sb
                    )
                    nc.vector.tensor_copy(v1_sb[:, :, kti, 0:64], tp)

            for ch in range(NPC):
                ssl = slice(ch * PC, (ch + 1) * PC)
                h_sb = hp.tile([128, KT, PC], f16)
                nc.sync.dma_start(h_sb, hR[:, ch, :, :])
                for wi, (w_sb, dst) in enumerate(
                    [(wq_sb, qT_sb), (wk_sb, kp_sb), (wv_sb, None)]
                ):
                    ps = pps.tile([128, PC], f32, tag="ps", name=f"ps_{ch}_{wi}")
                    for k in range(KT):
                        nc.tensor.matmul(
                            ps,
                            w_sb[:, k, :],
                            h_sb[:, k, :],
                            start=(k == 0),
                            stop=(k == KT - 1),
                        )
                    if pend:
                        pend.pop(0)()
                    if dst is not None:
                        # stage with bias on ACT (psum -> fp16 sbuf)
                        stg = rp.tile([128, PC], f16, tag="stg", name=f"stg_{ch}_{wi}")
                        nc.scalar.activation(
                            stg, ps, Ident, bias=b_sb[:, wi : wi + 1]
                        )
                        pend.append(
                            lambda ch=ch, wi=wi, stg=stg, dst=dst: rope_tail(
                                ch, wi, stg, dst
                            )
                        )
                    else:
                        stgv = rp.tile([128, PC], f16, tag="stgv", name=f"stgv_{ch}")
                        nc.scalar.activation(stgv, ps, Ident)
                        pend.append(lambda ch=ch, stgv=stgv: v_tail(ch, stgv))
                # chunk-0 attention scores for the key tiles this proj chunk
                # just produced: the exp work rides the otherwise idle
                # ACT/DVE capacity of the prologue
                for i in range(4 * ch, 4 * ch + 4):
                    emit_qk_exp(ssP, pts0, 0, i)
            for w in pend:
                w()

        # ---- attention + o_proj ----
        # PV runs one full chunk behind QK/exp: chunk c's loop emits QK/exp
        # for chunk c and PV for chunk c-1 from the persisted pt ring, so the
        # epilogue (den -> rec -> normalize) of each chunk has a whole chunk
        # of slack before its cx slot is needed again, and chunk 0's QK/exp
        # were already emitted in the prologue.
        with ExitStack() as ph2:
            ssp = ph2.enter_context(tc.tile_pool(name="ssp", bufs=3, space="PSUM"))
            cxp = ph2.enter_context(tc.tile_pool(name="cxp", bufs=1, space="PSUM"))
            obp = ph2.enter_context(tc.tile_pool(name="obp", bufs=4))
            epp = ph2.enter_context(tc.tile_pool(name="epp", bufs=2))

            def emit_oproj_pair(c, sq):
                # one [128 q, 1024 hid] output row-block of chunk c's o_proj:
                # two matmuls into the two banks of one ss slot, one staging
                # copy, one DMA
                r0 = c * CH + sq * 128
                ops = ssp.tile([128, 2 * CH], f32, tag="ss", name=f"op_{c}_{sq}")
                for nz in range(2):
                    nc.tensor.matmul(
                        ops[:, nz * 512 : (nz + 1) * 512],
                        ctx_sb[:, r0 : r0 + 128],
                        wo_sb[:, nz * 512 : (nz + 1) * 512],
                        start=True,
                        stop=True,
                    )
                ob = obp.tile([128, 1024], f16, tag="ob", name=f"ob_{c}_{sq}")
                # psum->sbuf fp16 staging; ACT takes most (DVE exp is pricier)
                if sq % 4 == 1:
                    nc.vector.tensor_copy(ob, ops)
                else:
                    nc.scalar.activation(ob, ops, Ident)
                nc.sync.dma_start(out[r0 : r0 + 128, :], ob)

            def emit_warmer(tag, n=10):
                # dense burst of throwaway M=1 matmuls: HAM un-throttles only
                # after a sustained-busy window. Alternate between the two
                # banks of one ss slot so same-bank WAW doesn't serialize.
                wps = ssp.tile([128, 2 * CH], f32, tag="ss", name=f"warm_{tag}")
                for j in range(n):
                    nc.tensor.matmul(
                        wps[0:1, (j % 2) * CH : (j % 2) * CH + CH],
                        kp_sb[:, 0:1],
                        qT_sb[:, 0:CH],
                        start=True,
                        stop=True,
                        skip_group_check=True,
                    )

            def emit_pv(cx, pts_list, i):
                for hh in range(2):
                    nc.tensor.matmul(
                        cx[:, hh, :],
                        v1_sb[:, hh, i, :],
                        pts_list[i][:, hh * CH : (hh + 1) * CH],
                        start=(i == 0),
                        stop=(i == SK - 1),
                    )

            # the epilogue is split so the rb broadcast matmul (which waits
            # on the Ln->Exp chain) never blocks the in-order PE queue at a
            # chunk boundary: part A (ACT/DVE work) is emitted at the end of
            # the producing loop, part B (rb + normalize) three iterations
            # into the next loop.
            recs = {}

            def emit_epilogue_a(c, cx):
                # den -> Ln straight from psum while DVE stages the context;
                # both heads in single wide instructions
                lnb = epp.tile([1, 2 * CH], f32, tag="lnb", name=f"lnb_{c}")
                nc.scalar.activation(lnb, cx[64:65, :, :], Ln)
                stage = epp.tile([64, 2, CH], f16, tag="stage", name=f"stage_{c}")
                nc.vector.tensor_copy(stage, cx[0:64, :, :])
                rec = epp.tile([1, 2 * CH], f16, tag="rec", name=f"rec_{c}")
                # rec = 4096/den keeps fp16 in normal range; host undoes it
                nc.scalar.activation(rec, lnb, Exp, scale=-1.0, bias=ln4096_sb[:, :])
                recs[c] = (rec, stage)

            def emit_epilogue_b(c):
                cs0 = c * CH
                rec, stage = recs.pop(c)
                rb = ssp.tile([128, 2 * CH], f32, tag="ss", name=f"rb_{c}")
                for hh in range(2):
                    nc.tensor.matmul(
                        rb[0:64, hh * CH : (hh + 1) * CH],
                        ones_sb,
                        rec[:, hh * CH : (hh + 1) * CH],
                        start=True,
                        stop=True,
                    )
                for hh in range(2):
                    hsl = slice(hh * 64, hh * 64 + 64)
                    nc.vector.tensor_mul(
                        ctx_sb[hsl, cs0 : cs0 + CH],
                        stage[:, hh, :],
                        rb[0:64, hh * CH : (hh + 1) * CH],
                    )

            PVL = 2  # intra-loop PV lag: epilogue A of the previous chunk
            # releases the cx slot before the first lagged PV needs it

            pts_prev = pts0
            for c in range(1, NCH):
                cx = cxp.tile([65, 2, CH], f32, tag="cx", name=f"cx_{c - 1}")
                pts_cur = [None] * SK
                for i in range(SK):
                    emit_qk_exp(ssp, pts_cur, c, i)
                    if i >= PVL:
                        emit_pv(cx, pts_prev, i - PVL)
                    if i == 3 and c >= 2:
                        emit_epilogue_b(c - 2)
                    if i == 10 or i == 20:
                        if c >= 2:
                            for sq in range(2 * (i == 20), 2 * (i == 20) + 2):
                                emit_oproj_pair(c - 2, sq)
                        else:
                            emit_warmer(f"w{c}_{i}")
                for i in range(SK - PVL, SK):
                    emit_pv(cx, pts_prev, i)
                emit_epilogue_a(c - 1, cx)
                pts_prev = pts_cur
            # drain: PV + epilogue + o_proj of the last two chunks
            cx = cxp.tile([65, 2, CH], f32, tag="cx", name=f"cx_{NCH - 1}")
            for i in range(SK):
                if i >= PVL:
                    emit_pv(cx, pts_prev, i - PVL)
                if i == 3:
                    emit_epilogue_b(NCH - 2)
                if i == 10 or i == 20:
                    for sq in range(2 * (i == 20), 2 * (i == 20) + 2):
                        emit_oproj_pair(NCH - 2, sq)
            for i in range(SK - PVL, SK):
                emit_pv(cx, pts_prev, i)
            emit_epilogue_a(NCH - 1, cx)
            emit_epilogue_b(NCH - 1)
            for sq in range(4):
                emit_oproj_pair(NCH - 1, sq)
    return nc


def _legalize_sync_waits(nc, max_waits=1):
    """Cap sync waits per instruction for this container's walrus build.

    The bundled walrus encodes a limited number of sync-wait commands per
    instruction ("Too many sync wait commands" codegen error), while Tile
    attaches one wait per logical processor where needed. An attached wait
    is equivalent to a standalone preceding wait on the same engine (that
    is exactly what raw-bass `wait_ge` emits: a pure-wait
    InstEventSemaphore), so hoist the excess waits onto EventSemaphore
    instructions inserted right before the offender.
    """
    from concourse import mybir

    n_fixed = 0
    for fn in nc.m.functions:
        for b in fn.blocks:
            insts = b.instructions
            idx = 0
            while idx < len(insts):
                inst = insts[idx]
                si = inst.sync_info
                waits = list(si.on_wait) if si and si.on_wait else []
                if len(waits) > max_waits:
                    updates = list(si.on_update) if si and si.on_update else []
                    pre, keep = waits[: -max_waits], waits[-max_waits:]
                    clones = []
                    for j, w in enumerate(pre):
                        clones.append(
                            mybir.InstEventSemaphore(
                                name=f"{inst.name}_sw{j}",
                                engine=inst.engine,
                                ins=[],
                                outs=[],
                                sync_info=mybir.SyncInfo(on_wait=[w], on_update=[]),
                            )
                        )
                    inst.sync_info = mybir.SyncInfo(on_wait=keep, on_update=updates)
                    for j, clone in enumerate(clones):
                        insts.insert(idx + j, clone)
                        try:
                            nc.inst_map[clone.name] = clone
                        except Exception:
                            pass
                    idx += len(clones)
                    n_fixed += 1
                idx += 1
    return n_fixed


MM_DT = "float16"


def get_nc(S=SEQ, mm_dt=MM_DT):
    key = S
    if key not in _NC_CACHE:
        nc = _build_nc(S)
        _legalize_sync_waits(nc)
        _NC_CACHE[key] = nc
    return _NC_CACHE[key]


def make_in_maps(h, cos, sin, wq, bq, wk, bk, wv, bv, wo):
    """Host-side shard prep. h [B,S,HID] -> per-core input dict."""
    f16 = np.float16
    h = np.asarray(h, dtype=np.float32)
    S = h.shape[1]
    PC, KT = 512, HIDDEN // 128
    NPC = S // PC
    # hR[p, ch, ko, s'] = h[ch*PC+s', ko*128+p]: one contiguous 8KB
    # descriptor per partition per chunk DMA
    hR = np.ascontiguousarray(
        h[0].reshape(NPC, PC, KT, 128).transpose(3, 0, 2, 1).astype(f16)
    )
    cosT = np.ascontiguousarray(np.asarray(cos, np.float32).T).astype(f16)
    sinT = np.ascontiguousarray(np.asarray(sin, np.float32).T).astype(f16)
    wq = np.asarray(wq, dtype=np.float32)
    wk = np.asarray(wk, dtype=np.float32)
    wv = np.asarray(wv, dtype=np.float32)
    wo = np.asarray(wo, dtype=np.float32)
    bq = np.asarray(bq, dtype=np.float32)
    bk = np.asarray(bk, dtype=np.float32)

    def wR(w, fs):
        # wR[p, ko, f] = w[fs][f, ko*128+p]
        return np.ascontiguousarray(
            w[fs, :].T.reshape(KT, 128, FPC).transpose(1, 0, 2).astype(f16)
        )

    in_maps = []
    for c in range(NCORES):
        fs = slice(c * FPC, (c + 1) * FPC)
        in_maps.append(
            {
                "hR": hR,
                "wqR": wR(wq, fs),
                "wkR": wR(wk, fs),
                "wvR": wR(wv, fs),
                "bqk": np.ascontiguousarray(
                    np.stack([bq[fs], bk[fs]], axis=1).astype(np.float32)
                ),
                "woT": np.ascontiguousarray(wo[:, fs].T).astype(f16),
                "cosT": cosT,
                "sinT": sinT,
            }
        )
    return in_maps


def kernel(h, mask, cos, sin, wq, bq, wk, bk, wv, bv, wo, bo, **_unused):
    # mask is all-ones per the problem spec; post-softmax where(mask==0) is a no-op.
    from concourse.bass_utils import run_bass_kernel_spmd

    h = np.asarray(h, dtype=np.float32)
    S = h.shape[1]
    nc = get_nc(S)
    in_maps = make_in_maps(h, cos, sin, wq, bq, wk, bk, wv, bv, wo)
    res = run_bass_kernel_spmd(nc, in_maps, core_ids=list(range(NCORES)))
    acc = np.zeros((S, HIDDEN), dtype=np.float64)
    for r in res.results:
        acc += r["out"].astype(np.float64)
    acc /= 4096.0
    bo_eff = np.asarray(bo, np.float64) + np.asarray(wo, np.float64) @ np.asarray(
        bv, np.float64
    )
    acc += bo_eff[None, :]
    return acc[None].astype(np.float32)


# revision 36
# speedup vs baseline: 1.0513x; 1.0247x over previous
"""Trainium2 Bass kernel for MHA with RoPE (dense transformer block).

Problem shapes: h [1, 4096, 1024], 16 heads x 64 dim, full (non-causal)
softmax attention, post-softmax all-ones mask (identity), torch-Linear
projections with bias.

Sharding: head-parallel across 8 cores (2 heads/core). v2 design:

Prologue (projections, ~50us):
  - q/k/v projections from fp16 hT with weights stationary.
  - RoPE via PE rotation matmul: qs = R*(q+b) where R is the
    rotate-half permutation as an fp16 [128,128] matrix; then on DVE
    q' = (q+b) o cos + qs o sin (bias folded into ACT staging copy).
  - v transposed to key-major via fp16 PE transpose; bias bv is folded
    into bo on the host (softmax rows sum to 1 exactly).

Attention (PE/ACT/DVE balanced, ~185us):
  - per 512-query chunk x 32 key tiles: row-tiled QK pair (K=64 per
    head at tile_position (0,0)/(64,0), both heads concurrent, no
    zero-padding), one [128,1024] psum score tile per iteration.
  - exp alternates between ACT (table exp, scale=1/8) and DVE
    (Schraudolph: fp16 bits = int16(score*A + B), one tensor_scalar,
    bitcast free) halving the softmax wall time.
  - PV with a ones-row appended to v (denominator for free, M=65).
  - normalize with rec = exp(-ln(den) + ln 4096) on ACT (scaled to
    stay in fp16 normal range; host divides by 4096), broadcast via
    K=1 ones matmul, applied by DVE.
  - o_proj partials DMA'd straight from PSUM to DRAM fp32.

Host sums the 8 partial outputs, divides by 4096, adds bo + wo@bv.
"""

import math

import numpy as np

HIDDEN = 1024
HEADS = 16
HEAD_DIM = 64
SEQ = 4096
NCORES = 8
FPC = 128  # features per core = 2 heads x 64

# Schraudolph fp16-exp constants: fp16bits(e^(s/8)) ~= int16(s*EXP_A + EXP_B)
EXP_A = 1024.0 * 0.125 / math.log(2.0)
EXP_B = 15360.0 - 44.0

_NC_CACHE = {}


def _build_nc(S=SEQ):
    import concourse.bass as bass
    import concourse.tile as tile
    from concourse import mybir
    from contextlib import ExitStack

    f32 = mybir.dt.float32
    f16 = mybir.dt.float16
    i16 = mybir.dt.int16
    Exp = mybir.ActivationFunctionType.Exp
    Ln = mybir.ActivationFunctionType.Ln
    Ident = mybir.ActivationFunctionType.Identity
    Mult = mybir.AluOpType.mult
    Add = mybir.AluOpType.add

    D = HEAD_DIM
    HID = HIDDEN
    KT = HID // 128  # hidden contraction tiles
    PC = 512         # projection seq chunk
    NPC = S // PC
    CH = 512         # attention query chunk
    NCH = S // CH
    SK = S // 128    # key tiles
    PVLAG = 4        # software-pipeline lag of PV behind QK/exp

    nc = bass.Bass(trn_type="TRN2")

    # host pre-arranges h and the q/k/v weights partition-major so every
    # input DMA is one contiguous descriptor per partition (the naive
    # "(ko p) s" rearrange yields 1024 scattered 256B descriptors and a
    # ~20us transfer)
    hR = nc.dram_tensor("hR", [128, NPC, KT, PC], f16, kind="ExternalInput")
    wqR = nc.dram_tensor("wqR", [128, KT, FPC], f16, kind="ExternalInput")
    wkR = nc.dram_tensor("wkR", [128, KT, FPC], f16, kind="ExternalInput")
    wvR = nc.dram_tensor("wvR", [128, KT, FPC], f16, kind="ExternalInput")
    bqk = nc.dram_tensor("bqk", [FPC, 2], f32, kind="ExternalInput")
    woT = nc.dram_tensor("woT", [FPC, HID], f16, kind="ExternalInput")
    cosT = nc.dram_tensor("cosT", [D, S], f16, kind="ExternalInput")
    sinT = nc.dram_tensor("sinT", [D, S], f16, kind="ExternalInput")
    out = nc.dram_tensor("out", [S, HID], f16, kind="ExternalOutput")

    # rotate-half as a matmul: qs = R @ q with R = blockdiag([[0,-I],[I,0]]).
    # matmul computes lhsT.T @ rhs so we feed R^T = blockdiag([[0,I],[-I,0]]).
    rotT_np = np.zeros((FPC, FPC), dtype=np.float16)
    for hh in range(2):
        o = hh * 64
        for j in range(32):
            rotT_np[o + 32 + j, o + j] = -1.0
            rotT_np[o + j, o + 32 + j] = 1.0
    rotT_dram = nc.inline_tensor(rotT_np, name="rotT")
    ident_np = np.eye(128, dtype=np.float16)
    ident_dram = nc.inline_tensor(ident_np, name="ident16")

    with tile.TileContext(nc) as tc, ExitStack() as top:
        sing = top.enter_context(tc.tile_pool(name="sing", bufs=1))

        # input DMAs are spread across the engines' DGE queues so the first
        # h chunk isn't serialized behind weights/tables on one queue: h
        # chunks own the sync queue, weights go via scalar, tables via
        # vector/gpsimd.
        wq_sb = sing.tile([128, KT, FPC], f16)
        wk_sb = sing.tile([128, KT, FPC], f16)
        wv_sb = sing.tile([128, KT, FPC], f16)
        nc.scalar.dma_start(wq_sb, wqR[:, :, :])
        nc.scalar.dma_start(wk_sb, wkR[:, :, :])
        nc.scalar.dma_start(wv_sb, wvR[:, :, :])
        cos_sb = sing.tile([128, S], f16)
        sin_sb = sing.tile([128, S], f16)
        nc.scalar.dma_start(cos_sb[0:64, :], cosT[:, :])
        nc.scalar.dma_start(sin_sb[0:64, :], sinT[:, :])
        nc.scalar.dma_start(cos_sb[64:128, :], cosT[:, :])
        nc.scalar.dma_start(sin_sb[64:128, :], sinT[:, :])
        wo_sb = sing.tile([FPC, HID], f16)
        nc.scalar.dma_start(wo_sb, woT[:, :])
        b_sb = sing.tile([FPC, 2], f32)
        nc.gpsimd.dma_start(b_sb, bqk[:, :])
        rot_sb = sing.tile([FPC, FPC], f16)
        nc.gpsimd.dma_start(rot_sb, rotT_dram[:, :])
        ident_sb = sing.tile([128, 128], f16)
        nc.gpsimd.dma_start(ident_sb, ident_dram[:, :])
        ones_sb = sing.tile([1, 64], f16)
        nc.vector.memset(ones_sb, 1.0)
        ln4096_sb = sing.tile([1, 1], f32)
        nc.vector.memset(ln4096_sb, math.log(4096.0))

        # warm the natural_log_exp table set during the prologue so the
        # first attention exp doesn't eat the ~2.7us ACT_TABLE_LOAD
        warm_sb = sing.tile([1, 1], f32)
        nc.scalar.activation(warm_sb, ln4096_sb, Ln)
        nc.scalar.activation(warm_sb, warm_sb, Exp)

        qT_sb = sing.tile([128, S], f16)
        kp_sb = sing.tile([128, S], f16)
        # v1[:, hh, i, 0:64] = v tile (keys-major); [.., 64] = ones row so the
        # PV matmul also produces the softmax denominator.
        v1_sb = sing.tile([128, 2, SK, 65], f16)
        nc.vector.memset(v1_sb, 1.0)
        ctx_sb = sing.tile([128, S], f16)

        # pt tiles span both phases: chunk 0's probabilities are computed
        # during the projection prologue (exp-ahead) and consumed by PV in
        # the attention phase; the ring is deep enough to hold one full
        # chunk (32) plus the next chunk's in-flight tiles.
        ptp = top.enter_context(tc.tile_pool(name="ptp", bufs=40))

        def emit_qk_exp(pool, pts_list, c, i):
            cs0 = c * CH
            ksl = slice(i * 128, (i + 1) * 128)
            csl = slice(cs0, cs0 + CH)
            ss = pool.tile([128, 2 * CH], f32, tag="ss", name=f"ss_{c}_{i}")
            for hh in range(2):
                psl = slice(hh * 64, hh * 64 + 64)
                nc.tensor.matmul(
                    ss[:, hh * CH : (hh + 1) * CH],
                    kp_sb[psl, ksl],
                    qT_sb[psl, csl],
                    start=True,
                    stop=True,
                )
            pt = ptp.tile([128, 2 * CH], f16, tag="pt", name=f"pt_{c}_{i}")
            # strict alternation: back-to-back same-engine exps serialize
            # and stall the ss ring
            if i % 2 == 0:
                nc.scalar.activation(pt, ss, Exp, scale=0.125)
            else:
                nc.vector.tensor_scalar(
                    pt[:, :].bitcast(i16), ss, EXP_A, EXP_B, Mult, Add
                )
            pts_list[i] = pt

        pts0 = [None] * SK

        # ---- projections + RoPE + v transpose (+ chunk-0 QK/exp) ----
        with ExitStack() as ph1:
            hp = ph1.enter_context(tc.tile_pool(name="hp", bufs=2))
            rp = ph1.enter_context(tc.tile_pool(name="rope", bufs=8))
            pps = ph1.enter_context(tc.tile_pool(name="pps", bufs=2, space="PSUM"))
            qsp = ph1.enter_context(tc.tile_pool(name="qsp", bufs=1, space="PSUM"))
            tps = ph1.enter_context(tc.tile_pool(name="tps", bufs=1, space="PSUM"))
            ssP = ph1.enter_context(tc.tile_pool(name="ssP", bufs=2, space="PSUM"))
            # Software-pipelined: each projection group's PE epilogue (the
            # rotation matmul / v transposes, which wait on an ACT staging
            # copy) is emitted under the NEXT group's matmul stream so the
            # PE never stalls on ACT latency.
            pend = []

            def rope_tail(ch, wi, stg, dst):
                ssl = slice(ch * PC, (ch + 1) * PC)
                qs = qsp.tile([128, PC], f32, tag="qs", name=f"qs_{ch}_{wi}")
                nc.tensor.matmul(qs, rot_sb, stg, start=True, stop=True)
                t1 = rp.tile([128, PC], f16, tag="t1", name=f"t1_{ch}_{wi}")
                nc.vector.tensor_mul(t1, stg, cos_sb[:, ssl])
                t2 = rp.tile([128, PC], f16, tag="t2", name=f"t2_{ch}_{wi}")
                nc.vector.tensor_mul(t2, qs, sin_sb[:, ssl])
                nc.vector.tensor_add(dst[:, ssl], t1, t2)

            def v_tail(ch, stgv):
                for st in range(PC // 128):
                    kti = ch * (PC // 128) + st
                    tp = tps.tile([128, 128], f16, tag="tp", name=f"tp_{ch}_{st}")
                    nc.tensor.transpose(
                        tp, stgv[:, st * 128 : (st + 1) * 128], ident_sb
                    )
                    nc.vector.tensor_copy(v1_sb[:, :, kti, 0:64], tp)

            for ch in range(NPC):
                ssl = slice(ch * PC, (ch + 1) * PC)
                h_sb = hp.tile([128, KT, PC], f16)
                nc.sync.dma_start(h_sb, hR[:, ch, :, :])
                for wi, (w_sb, dst) in enumerate(
                    [(wq_sb, qT_sb), (wk_sb, kp_sb), (wv_sb, None)]
                ):
                    ps = pps.tile([128, PC], f32, tag="ps", name=f"ps_{ch}_{wi}")
                    for k in range(KT):
                        nc.tensor.matmul(
                            ps,
                            w_sb[:, k, :],
                            h_sb[:, k, :],
                            start=(k == 0),
                            stop=(k == KT - 1),
                        )
                    if pend:
                        pend.pop(0)()
                    if dst is not None:
                        # stage with bias on ACT (psum -> fp16 sbuf)
                        stg = rp.tile([128, PC], f16, tag="stg", name=f"stg_{ch}_{wi}")
                        nc.scalar.activation(
                            stg, ps, Ident, bias=b_sb[:, wi : wi + 1]
                        )
                        pend.append(
                            lambda ch=ch, wi=wi, stg=stg, dst=dst: rope_tail(
                                ch, wi, stg, dst
                            )
                        )
                    else:
                        stgv = rp.tile([128, PC], f16, tag="stgv", name=f"stgv_{ch}")
                        nc.scalar.activation(stgv, ps, Ident)
                        pend.append(lambda ch=ch, stgv=stgv: v_tail(ch, stgv))
                # chunk-0 attention scores for the key tiles this proj chunk
                # just produced: the exp work rides the otherwise idle
                # ACT/DVE capacity of the prologue
                for i in range(4 * ch, 4 * ch + 4):
                    emit_qk_exp(ssP, pts0, 0, i)
            for w in pend:
                w()

        # ---- attention + o_proj ----
        # PV runs one full chunk behind QK/exp: chunk c's loop emits QK/exp
        # for chunk c and PV for chunk c-1 from the persisted pt ring, so the
        # epilogue (den -> rec -> normalize) of each chunk has a whole chunk
        # of slack before its cx slot is needed again, and chunk 0's QK/exp
        # were already emitted in the prologue.
        with ExitStack() as ph2:
            ssp = ph2.enter_context(tc.tile_pool(name="ssp", bufs=3, space="PSUM"))
            cxp = ph2.enter_context(tc.tile_pool(name="cxp", bufs=1, space="PSUM"))
            obp = ph2.enter_context(tc.tile_pool(name="obp", bufs=4))
            epp = ph2.enter_context(tc.tile_pool(name="epp", bufs=2))

            def emit_oproj_pair(c, sq):
                # one [128 q, 1024 hid] output row-block of chunk c's o_proj:
                # two matmuls into the two banks of one ss slot, one staging
                # copy, one DMA
                r0 = c * CH + sq * 128
                ops = ssp.tile([128, 2 * CH], f32, tag="ss", name=f"op_{c}_{sq}")
                for nz in range(2):
                    nc.tensor.matmul(
                        ops[:, nz * 512 : (nz + 1) * 512],
                        ctx_sb[:, r0 : r0 + 128],
                        wo_sb[:, nz * 512 : (nz + 1) * 512],
                        start=True,
                        stop=True,
                    )
                ob = obp.tile([128, 1024], f16, tag="ob", name=f"ob_{c}_{sq}")
                # staging split into halves so no single ACT/DVE queue
                # insertion overflows the slack between consecutive exps;
                # ACT takes most halves (DVE's exp is pricier)
                for nz in range(2):
                    osl = slice(nz * 512, (nz + 1) * 512)
                    if sq % 4 == 1 and nz == 0:
                        nc.vector.tensor_copy(ob[:, osl], ops[:, osl])
                    else:
                        nc.scalar.activation(ob[:, osl], ops[:, osl], Ident)
                nc.sync.dma_start(out[r0 : r0 + 128, :], ob)

            def emit_warmer(tag, n=10):
                # dense burst of throwaway M=1 matmuls: HAM un-throttles only
                # after a sustained-busy window. Alternate between the two
                # banks of one ss slot so same-bank WAW doesn't serialize.
                wps = ssp.tile([128, 2 * CH], f32, tag="ss", name=f"warm_{tag}")
                for j in range(n):
                    nc.tensor.matmul(
                        wps[0:1, (j % 2) * CH : (j % 2) * CH + CH],
                        kp_sb[:, 0:1],
                        qT_sb[:, 0:CH],
                        start=True,
                        stop=True,
                        skip_group_check=True,
                    )

            def emit_pv(cx, pts_list, i):
                for hh in range(2):
                    nc.tensor.matmul(
                        cx[:, hh, :],
                        v1_sb[:, hh, i, :],
                        pts_list[i][:, hh * CH : (hh + 1) * CH],
                        start=(i == 0),
                        stop=(i == SK - 1),
                    )

            # The epilogue of chunk c-1 is sliced into small pieces spread
            # over the next loop's iterations so no piece blocks the in-order
            # PE queue at a boundary and no ACT/DVE insertion lands as one
            # big blob in front of a ring-critical exp:
            #   loop end: stage copy (DVE; releases cx with Ln below)
            #   iter 1:   Ln(den) on ACT
            #   iter 4:   rec = Exp(-ln den + ln 4096) on ACT
            #   iter 7:   rb broadcast (PE) + normalize muls (DVE)
            epi = {}

            def emit_stage(c, cx):
                stage = epp.tile([64, 2, CH], f16, tag="stage", name=f"stage_{c}")
                nc.vector.tensor_copy(stage, cx[0:64, :, :])
                epi[c] = {"cx": cx, "stage": stage}

            def emit_ln(c):
                lnb = epp.tile([1, 2 * CH], f32, tag="lnb", name=f"lnb_{c}")
                nc.scalar.activation(lnb, epi[c]["cx"][64:65, :, :], Ln)
                epi[c]["lnb"] = lnb

            def emit_rec(c):
                rec = epp.tile([1, 2 * CH], f16, tag="rec", name=f"rec_{c}")
                # rec = 4096/den keeps fp16 in normal range; host undoes it
                nc.scalar.activation(
                    rec, epi[c]["lnb"], Exp, scale=-1.0, bias=ln4096_sb[:, :]
                )
                epi[c]["rec"] = rec

            def emit_rbnorm(c):
                cs0 = c * CH
                e = epi.pop(c)
                rb = ssp.tile([128, 2 * CH], f32, tag="ss", name=f"rb_{c}")
                for hh in range(2):
                    nc.tensor.matmul(
                        rb[0:64, hh * CH : (hh + 1) * CH],
                        ones_sb,
                        e["rec"][:, hh * CH : (hh + 1) * CH],
                        start=True,
                        stop=True,
                    )
                for hh in range(2):
                    hsl = slice(hh * 64, hh * 64 + 64)
                    nc.vector.tensor_mul(
                        ctx_sb[hsl, cs0 : cs0 + CH],
                        e["stage"][:, hh, :],
                        rb[0:64, hh * CH : (hh + 1) * CH],
                    )

            PVL = 2  # intra-loop PV lag: the previous chunk's stage/Ln
            # release the cx slot before the first lagged PV needs it

            def body(c, i, pts_cur, cx, pts_prev):
                if pts_cur is not None:
                    emit_qk_exp(ssp, pts_cur, c, i)
                if i >= PVL:
                    emit_pv(cx, pts_prev, i - PVL)
                # epilogue pieces + o_proj for chunk c-2 (whose PVs finished
                # at the end of loop c-1, where its stage copy was emitted)
                if c >= 2:
                    if i == 1:
                        emit_ln(c - 2)
                    elif i == 4:
                        emit_rec(c - 2)
                    elif i == 7:
                        emit_rbnorm(c - 2)
                    elif i in (12, 16, 20, 24):
                        emit_oproj_pair(c - 2, (i - 12) // 4)
                elif i in (12, 20):
                    emit_warmer(f"w{c}_{i}")

            pts_prev = pts0
            for c in range(1, NCH):
                cx = cxp.tile([65, 2, CH], f32, tag="cx", name=f"cx_{c - 1}")
                pts_cur = [None] * SK
                for i in range(SK):
                    body(c, i, pts_cur, cx, pts_prev)
                for i in range(SK - PVL, SK):
                    emit_pv(cx, pts_prev, i)
                emit_stage(c - 1, cx)
                pts_prev = pts_cur
            # drain: PV + epilogue + o_proj of the last two chunks
            cx = cxp.tile([65, 2, CH], f32, tag="cx", name=f"cx_{NCH - 1}")
            for i in range(SK):
                body(NCH, i, None, cx, pts_prev)
            for i in range(SK - PVL, SK):
                emit_pv(cx, pts_prev, i)
            emit_stage(NCH - 1, cx)
            emit_ln(NCH - 1)
            emit_rec(NCH - 1)
            emit_rbnorm(NCH - 1)
            for sq in range(4):
                emit_oproj_pair(NCH - 1, sq)
    return nc


def _legalize_sync_waits(nc, max_waits=1):
    """Cap sync waits per instruction for this container's walrus build.

    The bundled walrus encodes a limited number of sync-wait commands per
    instruction ("Too many sync wait commands" codegen error), while Tile
    attaches one wait per logical processor where needed. An attached wait
    is equivalent to a standalone preceding wait on the same engine (that
    is exactly what raw-bass `wait_ge` emits: a pure-wait
    InstEventSemaphore), so hoist the excess waits onto EventSemaphore
    instructions inserted right before the offender.
    """
    from concourse import mybir

    n_fixed = 0
    for fn in nc.m.functions:
        for b in fn.blocks:
            insts = b.instructions
            idx = 0
            while idx < len(insts):
                inst = insts[idx]
                si = inst.sync_info
                waits = list(si.on_wait) if si and si.on_wait else []
                if len(waits) > max_waits:
                    updates = list(si.on_update) if si and si.on_update else []
                    pre, keep = waits[: -max_waits], waits[-max_waits:]
                    clones = []
                    for j, w in enumerate(pre):
                        clones.append(
                            mybir.InstEventSemaphore(
                                name=f"{inst.name}_sw{j}",
                                engine=inst.engine,
                                ins=[],
                                outs=[],
                                sync_info=mybir.SyncInfo(on_wait=[w], on_update=[]),
                            )
                        )
                    inst.sync_info = mybir.SyncInfo(on_wait=keep, on_update=updates)
                    for j, clone in enumerate(clones):
                        insts.insert(idx + j, clone)
                        try:
                            nc.inst_map[clone.name] = clone
                        except Exception:
                            pass
                    idx += len(clones)
                    n_fixed += 1
                idx += 1
    return n_fixed


MM_DT = "float16"


def get_nc(S=SEQ, mm_dt=MM_DT):
    key = S
    if key not in _NC_CACHE:
        nc = _build_nc(S)
        _legalize_sync_waits(nc)
        _NC_CACHE[key] = nc
    return _NC_CACHE[key]


def make_in_maps(h, cos, sin, wq, bq, wk, bk, wv, bv, wo):
    """Host-side shard prep. h [B,S,HID] -> per-core input dict."""
    f16 = np.float16
    h = np.asarray(h, dtype=np.float32)
    S = h.shape[1]
    PC, KT = 512, HIDDEN // 128
    NPC = S // PC
    # hR[p, ch, ko, s'] = h[ch*PC+s', ko*128+p]: one contiguous 8KB
    # descriptor per partition per chunk DMA
    hR = np.ascontiguousarray(
        h[0].reshape(NPC, PC, KT, 128).transpose(3, 0, 2, 1).astype(f16)
    )
    cosT = np.ascontiguousarray(np.asarray(cos, np.float32).T).astype(f16)
    sinT = np.ascontiguousarray(np.asarray(sin, np.float32).T).astype(f16)
    wq = np.asarray(wq, dtype=np.float32)
    wk = np.asarray(wk, dtype=np.float32)
    wv = np.asarray(wv, dtype=np.float32)
    wo = np.asarray(wo, dtype=np.float32)
    bq = np.asarray(bq, dtype=np.float32)
    bk = np.asarray(bk, dtype=np.float32)

    def wR(w, fs):
        # wR[p, ko, f] = w[fs][f, ko*128+p]
        return np.ascontiguousarray(
            w[fs, :].T.reshape(KT, 128, FPC).transpose(1, 0, 2).astype(f16)
        )

    in_maps = []
    for c in range(NCORES):
        fs = slice(c * FPC, (c + 1) * FPC)
        in_maps.append(
            {
                "hR": hR,
                "wqR": wR(wq, fs),
                "wkR": wR(wk, fs),
                "wvR": wR(wv, fs),
                "bqk": np.ascontiguousarray(
                    np.stack([bq[fs], bk[fs]], axis=1).astype(np.float32)
                ),
                "woT": np.ascontiguousarray(wo[:, fs].T).astype(f16),
                "cosT": cosT,
                "sinT": sinT,
            }
        )
    return in_maps


def kernel(h, mask, cos, sin, wq, bq, wk, bk, wv, bv, wo, bo, **_unused):
    # mask is all-ones per the problem spec; post-softmax where(mask==0) is a no-op.
    from concourse.bass_utils import run_bass_kernel_spmd

    h = np.asarray(h, dtype=np.float32)
    S = h.shape[1]
    nc = get_nc(S)
    in_maps = make_in_maps(h, cos, sin, wq, bq, wk, bk, wv, bv, wo)
    res = run_bass_kernel_spmd(nc, in_maps, core_ids=list(range(NCORES)))
    acc = np.zeros((S, HIDDEN), dtype=np.float64)
    for r in res.results:
        acc += r["out"].astype(np.float64)
    acc /= 4096.0
    bo_eff = np.asarray(bo, np.float64) + np.asarray(wo, np.float64) @ np.asarray(
        bv, np.float64
    )
    acc += bo_eff[None, :]
    return acc[None].astype(np.float32)
